# revision 51
# baseline (speedup 1.0000x reference)
"""Multi-head attention (B=8, S=2048, E=1024, H=8, D=128) on 8 Trainium2 cores.

Strategy: data-parallel over batch (one batch element per core, no collectives).

v2 design (vs v1 baseline at ~740us; measured ~290-420us, tunnel-noise
dependent):
  - All SBUF activations/weights in bf16 (halves DMA + SBUF; matmul same rate).
  - Scores run as fp8e4 DoubleRow matmuls (0.5 cyc/col, 2x): kt8 holds
    [k8 | k - k8] in the two DoubleRow slots (error-corrected K), the rhs
    reads q8 twice via a stride-0 broadcast AP, so scores = (k8+ke)·q8 with
    only Q's fp8 quantization error (~0.95% final rel err, tol 2e-2).
  - Softmax row sums come off the PE: es tiles are accumulated elementwise on
    DVE (bf16, 2x mode), then one [128,128] ones-matmul per (head, s-block)
    reduces across partitions. Saves ~110us of PE time. (GpSimd was tried for
    these adds and for the bv add: its tensor ops are far slower on real HW
    than the cost model claims — a gpsimd tensor_scalar_add on the AT path
    cost +300us measured. Keep elementwise work on DVE.)
  - The V bias is algebraically folded out of the projection:
    softmax(s)@(v+bv) = pv/rs + bv, so bv lands as a per-partition DVE add
    on the normalized AT, saving the rank-1 bias matmuls on the PE.
  - exp(st) writes es directly as bf16 (Act cost is width-bound, not dtype).
  - x is loaded once per rep (bf16, 4 s-block slabs), not once per pair.
  - PE executes its queue in order, so projection chunks for head g+1 are
    emitted interleaved into head g's attention loop (software-pipelined so
    pv(t-1) follows st(t)); the row-sum finisher of each attention unit is
    deferred into the next unit; output-projection chunks are emitted per
    s-block as soon as the last head finishes it.
  - PSUM: st [128,1024]x2 (4 banks) + pv [128,512]x2 + acc [128,512]x2 = 8.
  - qk8=True (off by default) additionally runs Q/K projections as fully
    error-corrected fp8 DoubleRow chains; the cost model says -35us but on
    real HW DR projection chains are weight-load-bound and ~150us SLOWER.
"""

import numpy as np
from contextlib import ExitStack

import concourse.bass as bass
import concourse.tile as tile
from concourse import bacc, mybir
from concourse.bass_utils import run_bass_kernel_spmd

try:
    import ml_dtypes
    BF_NP = ml_dtypes.bfloat16
except ImportError:  # pragma: no cover
    BF_NP = None

B = 8
S = 2048
E = 1024
H = 8
D = 128
P = 128
EC = E // P          # 8 contraction chunks over embed
TC = S // P          # 16 t-chunks
NSB = S // 512       # 4 s-blocks of 512
SCALE = 1.0 / float(np.sqrt(D))
# qk8 mode: Wq/Wk/Wv (and bq/bv) are prescaled by WSC host-side so the fp8
# weight residual tensors clear the e4m3 subnormal floor (2^-9); the exp
# scale absorbs 1/WSC^2 and Wo absorbs the V-side 1/WSC.
WSC = 16.0

F32 = mybir.dt.float32
BF16 = mybir.dt.bfloat16
FP8 = mybir.dt.float8e4
DR = mybir.MatmulPerfMode.DoubleRow
EXP = mybir.ActivationFunctionType.Exp
COPY = mybir.ActivationFunctionType.Copy


def build_nc(reps=1, use_fp8=True, qk8=False):
    nc = bacc.Bacc(None)

    WoT = nc.dram_tensor("WoT", (E, E), BF16, kind="ExternalInput")
    bq2 = nc.dram_tensor("bq2", (P, H), F32, kind="ExternalInput")   # bq as [d, h]
    bv2 = nc.dram_tensor("bv2", (P, H), F32, kind="ExternalInput")   # bv as [d, h]
    bob = nc.dram_tensor("bob", (P, E), BF16, kind="ExternalInput")  # bo bcast
    out = nc.dram_tensor("out", (S, E), F32, kind="ExternalOutput")

    wo_r = WoT.rearrange("(hc p) e -> p hc e", p=P)
    xT = nc.dram_tensor("xT", (E, S), BF16, kind="ExternalInput")
    WvT = nc.dram_tensor("WvT", (E, E), BF16, kind="ExternalInput")
    x_r = xT.rearrange("(ec p) s -> p ec s", p=P)
    wv_r = WvT.rearrange("(ec p) d -> p ec d", p=P)
    if qk8:
        # Q/K/V projections as fp8 DoubleRow chains with full first-order
        # error correction: x·w ~ x8·w8 + x8·we + xe·w8 (xe/we are the fp8
        # quantization residuals; the dropped xe·we term is ~0.13%).
        xT8 = nc.dram_tensor("xT8", (E, S), FP8, kind="ExternalInput")
        xTe = nc.dram_tensor("xTe", (E, S), FP8, kind="ExternalInput")
        Wq8T = nc.dram_tensor("Wq8T", (E, E), FP8, kind="ExternalInput")
        WqeT = nc.dram_tensor("WqeT", (E, E), FP8, kind="ExternalInput")
        Wk8T = nc.dram_tensor("Wk8T", (E, E), FP8, kind="ExternalInput")
        WkeT = nc.dram_tensor("WkeT", (E, E), FP8, kind="ExternalInput")
        x8_r = xT8.rearrange("(ec p) s -> p ec s", p=P)
        xe_r = xTe.rearrange("(ec p) s -> p ec s", p=P)
        wq8_r = Wq8T.rearrange("(ec p) d -> p ec d", p=P)
        wqe_r = WqeT.rearrange("(ec p) d -> p ec d", p=P)
        wk8_r = Wk8T.rearrange("(ec p) d -> p ec d", p=P)
        wke_r = WkeT.rearrange("(ec p) d -> p ec d", p=P)
    else:
        WqT = nc.dram_tensor("WqT", (E, E), BF16, kind="ExternalInput")
        WkT = nc.dram_tensor("WkT", (E, E), BF16, kind="ExternalInput")
        wq_r = WqT.rearrange("(ec p) d -> p ec d", p=P)
        wk_r = WkT.rearrange("(ec p) d -> p ec d", p=P)

    KDT = FP8 if use_fp8 else BF16

    with tile.TileContext(nc) as tc:
        with ExitStack() as octx:
            const = octx.enter_context(tc.tile_pool(name="const", bufs=1))
            atp = octx.enter_context(tc.tile_pool(name="atp", bufs=1))

            bq_s = const.tile([P, H], F32)
            nc.sync.dma_start(out=bq_s, in_=bq2[:, :])
            bv_s = const.tile([P, H], F32)   # bv as [d, h]
            nc.sync.dma_start(out=bv_s, in_=bv2[:, :])
            bo_s = const.tile([P, E], BF16)   # bo broadcast to all partitions
            nc.sync.dma_start(out=bo_s, in_=bob[:, :])
            ones1 = const.tile([1, P], BF16)
            nc.vector.memset(ones1, 1.0)
            onesf = const.tile([P, P], BF16)
            nc.vector.memset(onesf, 1.0)

            AT = atp.tile([P, H, S], BF16)  # normalized A^T per head

            for _rep in range(reps):
                with ExitStack() as ctx:
                    xsp = ctx.enter_context(tc.tile_pool(name="xsp", bufs=1))
                    wop = ctx.enter_context(tc.tile_pool(name="wop", bufs=1))
                    wpool = ctx.enter_context(tc.tile_pool(name="wpool", bufs=2))
                    qkp = ctx.enter_context(tc.tile_pool(name="qkp", bufs=2))
                    vp = ctx.enter_context(tc.tile_pool(name="vp", bufs=2))
                    esp = ctx.enter_context(tc.tile_pool(name="esp", bufs=4 if qk8 else 5))
                    rp = ctx.enter_context(tc.tile_pool(name="rp", bufs=2))
                    rcpp = ctx.enter_context(tc.tile_pool(name="rcpp", bufs=2))
                    outp = ctx.enter_context(tc.tile_pool(name="outp", bufs=2))
                    stp = ctx.enter_context(tc.tile_pool(name="stp", bufs=2, space="PSUM"))
                    pvp = ctx.enter_context(tc.tile_pool(name="pvp", bufs=2, space="PSUM"))
                    accp = ctx.enter_context(tc.tile_pool(name="accp", bufs=2, space="PSUM"))

                    # per-head state (set by emit_w / proj chunks)
                    head_tiles = {}

                    def emit_w(pair):
                        h0 = 2 * pair
                        if qk8:
                            ws = []
                            for nm, src in (("wq8", wq8_r), ("wqe", wqe_r),
                                            ("wk8", wk8_r), ("wke", wke_r)):
                                pairw = []
                                for hh in range(2):
                                    t = wpool.tile([P, EC, D], FP8, tag=f"{nm}{hh}")
                                    nc.sync.dma_start(
                                        out=t,
                                        in_=src[:, :, (h0 + hh) * D:(h0 + hh + 1) * D])
                                    pairw.append(t)
                                ws.append(pairw)
                            wv01 = wpool.tile([P, EC, 2 * D], BF16, tag="wv01")
                            nc.sync.dma_start(out=wv01, in_=wv_r[:, :, h0 * D:(h0 + 2) * D])
                            head_tiles[("w", pair)] = (ws, wv01)
                        else:
                            wq0 = wpool.tile([P, EC, D], BF16, tag="wq0")
                            wq1 = wpool.tile([P, EC, D], BF16, tag="wq1")
                            wk0 = wpool.tile([P, EC, D], BF16, tag="wk0")
                            wk1 = wpool.tile([P, EC, D], BF16, tag="wk1")
                            wv01 = wpool.tile([P, EC, 2 * D], BF16, tag="wv01")
                            nc.sync.dma_start(out=wq0, in_=wq_r[:, :, h0 * D:(h0 + 1) * D])
                            nc.sync.dma_start(out=wq1, in_=wq_r[:, :, (h0 + 1) * D:(h0 + 2) * D])
                            nc.sync.dma_start(out=wk0, in_=wk_r[:, :, h0 * D:(h0 + 1) * D])
                            nc.sync.dma_start(out=wk1, in_=wk_r[:, :, (h0 + 1) * D:(h0 + 2) * D])
                            nc.sync.dma_start(out=wv01, in_=wv_r[:, :, h0 * D:(h0 + 2) * D])
                            head_tiles[("w", pair)] = ((wq0, wq1, wk0, wk1), wv01)

                    def qk_mm(acc, g, which, sb):
                        """Emit the matmul chain for one Q or K proj chunk."""
                        pair, hi = g // 2, g % 2
                        ws, _ = head_tiles[("w", pair)]
                        if qk8:
                            w8 = ws[0 if which == "q" else 2][hi]
                            we = ws[1 if which == "q" else 3][hi]
                            nec = EC // 2
                            chains = ((w8, xs8), (we, xs8), (w8, xse))
                            for ci, (w, xops) in enumerate(chains):
                                for ecp in range(nec):
                                    nc.tensor.matmul(
                                        acc, w[:, 2 * ecp:2 * ecp + 2, :],
                                        xops[sb][:, 2 * ecp:2 * ecp + 2, :],
                                        start=(ci == 0 and ecp == 0),
                                        stop=(ci == 2 and ecp == nec - 1),
                                        perf_mode=DR)
                        else:
                            w = ws[(0 if which == "q" else 2) + hi]
                            for ec in range(EC):
                                nc.tensor.matmul(
                                    acc, w[:, ec, :], xs[sb][:, ec, :],
                                    start=(ec == 0), stop=(ec == EC - 1))

                    def proj_chunks(g):
                        """Closures projecting head g's K, V, Q (12 chunks)."""
                        pair, hi = g // 2, g % 2
                        _, wv01 = head_tiles[("w", pair)]
                        if use_fp8:
                            qt = qkp.tile([P, S], FP8, tag=f"qt{hi}")
                            kt = qkp.tile([P, 2, S], FP8, tag=f"kt{hi}")
                        else:
                            qt = qkp.tile([P, S], BF16, tag=f"qt{hi}")
                            kt = qkp.tile([P, S], BF16, tag=f"kt{hi}")
                        vv = vp.tile([P, TC, D], BF16, tag=f"vv{hi}")
                        head_tiles[("qkv", g)] = (qt, kt, vv)
                        chunks = []

                        def k_chunk(sb):
                            def emit():
                                acc = accp.tile([P, 512], F32, tag="acc",
                                                name=f"k{g}_{sb}")
                                qk_mm(acc, g, "k", sb)
                                sl = slice(sb * 512, (sb + 1) * 512)
                                if use_fp8:
                                    nc.scalar.activation(kt[:, 0, sl], acc, COPY)
                                    nc.vector.tensor_sub(kt[:, 1, sl], acc,
                                                         kt[:, 0, sl])
                                else:
                                    nc.scalar.activation(kt[:, sl], acc, COPY)
                            return emit

                        def v_chunk(sb):
                            def emit():
                                acc = accp.tile([P, 512], F32, tag="acc",
                                                name=f"v{g}_{sb}")
                                # bv is NOT added here: softmax(s)·(v+bv) =
                                # pv/rs + bv, so it lands per-partition on the
                                # normalized AT (GpSimd, in the finisher).
                                dsl = slice(hi * D, (hi + 1) * D)
                                for tl in range(4):
                                    o = tl * P
                                    for ec in range(EC):
                                        nc.tensor.matmul(
                                            acc[:, o:o + P],
                                            xs[sb][:, ec, o:o + P],
                                            wv01[:, ec, dsl],
                                            start=(ec == 0), stop=(ec == EC - 1))
                                nc.vector.tensor_copy(vv[:, sb * 4:(sb + 1) * 4, :], acc)
                            return emit

                        def q_chunk(sb):
                            def emit():
                                acc = accp.tile([P, 512], F32, tag="acc",
                                                name=f"q{g}_{sb}")
                                qk_mm(acc, g, "q", sb)
                                nc.vector.tensor_scalar_add(
                                    qt[:, sb * 512:(sb + 1) * 512], acc,
                                    bq_s[:, g:g + 1])
                            return emit

                        for sb in range(NSB):
                            chunks.append(k_chunk(sb))
                        for sb in range(NSB):
                            chunks.append(v_chunk(sb))
                        for sb in range(NSB):
                            chunks.append(q_chunk(sb))
                        return chunks

                    def att_unit(g, sb):
                        """Emits the unit's matmuls; returns a finisher closure
                        (row-sum reduce + normalize) for the caller to emit
                        later so the PE never stalls on the DVE/Pool chain."""
                        qt, kt, vv = head_tiles[("qkv", g)]
                        s0 = sb * 512
                        pv = pvp.tile([P, 512], F32, tag="pv", name=f"pv{g}_{sb}")
                        if use_fp8:
                            qrhs = qt[:, s0:s0 + 512].unsqueeze(1).broadcast_to([P, 2, 512])
                        es_tiles = []
                        ra = rb = None

                        def emit_st(tcp):
                            st = stp.tile([P, 1024], F32, tag="st",
                                          name=f"st{g}_{sb}_{tcp}")
                            for j in range(2):
                                tc_i = tcp * 2 + j
                                if use_fp8:
                                    nc.tensor.matmul(
                                        st[:, j * 512:(j + 1) * 512],
                                        kt[:, :, tc_i * P:(tc_i + 1) * P],
                                        qrhs, start=True, stop=True,
                                        perf_mode=DR)
                                else:
                                    nc.tensor.matmul(
                                        st[:, j * 512:(j + 1) * 512],
                                        kt[:, tc_i * P:(tc_i + 1) * P],
                                        qt[:, s0:s0 + 512],
                                        start=True, stop=True)
                            es = esp.tile([P, 1024], BF16, tag="es",
                                          name=f"es{g}_{sb}_{tcp}")
                            nc.scalar.activation(
                                es, st, EXP,
                                scale=SCALE / (WSC * WSC) if qk8 else SCALE)
                            es_tiles.append(es)

                        def emit_pv(tcp):
                            es = es_tiles[tcp]
                            for j in range(2):
                                nc.tensor.matmul(
                                    pv, vv[:, tcp * 2 + j, :],
                                    es[:, j * 512:(j + 1) * 512],
                                    start=(tcp == 0 and j == 0),
                                    stop=(tcp == TC // 2 - 1 and j == 1))

                        def emit_rs(tcp):
                            # GpSimd accumulates es0..es3 (early, off DVE);
                            # DVE takes es4..es7 + merge (minimal tail chain).
                            nonlocal ra, rb
                            if tcp == 1:
                                rb = rp.tile([P, 1024], BF16, tag="rb",
                                             name=f"rb{g}_{sb}")
                                nc.vector.tensor_add(rb, es_tiles[0], es_tiles[1])
                            elif tcp in (2, 3):
                                nc.vector.tensor_add(rb, rb, es_tiles[tcp])
                            elif tcp == 5:
                                ra = rp.tile([P, 1024], BF16, tag="ra",
                                             name=f"ra{g}_{sb}")
                                nc.vector.tensor_add(ra, es_tiles[4], es_tiles[5])
                            elif tcp in (6, 7):
                                nc.vector.tensor_add(ra, ra, es_tiles[tcp])

                        # software pipeline: pv(t-1) emitted after st(t)/exp(t)
                        emit_st(0)
                        for tcp in range(1, TC // 2):
                            emit_st(tcp)
                            emit_pv(tcp - 1)
                            emit_rs(tcp)
                        emit_pv(TC // 2 - 1)

                        def finisher():
                            nc.vector.tensor_add(ra, ra, rb)
                            fold = rp.tile([P, 512], BF16, tag="fold",
                                           name=f"fd{g}_{sb}")
                            nc.vector.tensor_add(fold, ra[:, 0:512], ra[:, 512:1024])
                            rsr = accp.tile([P, 512], F32, tag="acc",
                                            name=f"rs{g}_{sb}")
                            nc.tensor.matmul(rsr, onesf[:, :], fold,
                                             start=True, stop=True)
                            rcp = rcpp.tile([P, 512], F32, tag="rcp",
                                            name=f"rc{g}_{sb}")
                            nc.vector.reciprocal(rcp, rsr)
                            nc.vector.tensor_mul(AT[:, g, s0:s0 + 512], pv, rcp)
                            nc.vector.tensor_scalar_add(
                                AT[:, g, s0:s0 + 512], AT[:, g, s0:s0 + 512],
                                bv_s[:, g:g + 1])
                        return finisher

                    def out_chunks_sb(sb):
                        for sc in range(sb * 4, (sb + 1) * 4):
                            for j in range(2):
                                acc = accp.tile([P, 512], F32, tag="acc",
                                                name=f"o{sc}_{j}")
                                for hc in range(H):
                                    nc.tensor.matmul(
                                        acc, AT[:, hc, sc * P:(sc + 1) * P],
                                        wo[:, hc, j * 512:(j + 1) * 512],
                                        start=(hc == 0), stop=(hc == H - 1))
                                ot = outp.tile([P, 512], F32, tag="ot",
                                               name=f"ot{sc}_{j}")
                                # bias folded into the eviction add
                                nc.vector.tensor_add(
                                    ot, acc, bo_s[:, j * 512:(j + 1) * 512])
                                nc.sync.dma_start(
                                    out=out[sc * P:(sc + 1) * P,
                                            j * 512:(j + 1) * 512],
                                    in_=ot)

                    # ---- main emission ----
                    # DMA order: first s-block of x + pair-0 weights first so
                    # the PE starts ASAP; wo (needed last) goes last.
                    xs = []
                    xs8 = []
                    xse = []

                    def load_x(sb):
                        sl = slice(sb * 512, (sb + 1) * 512)
                        xt = xsp.tile([P, EC, 512], BF16, tag=f"xs{sb}")
                        nc.sync.dma_start(out=xt, in_=x_r[:, :, sl])
                        xs.append(xt)
                        if qk8:
                            xt8 = xsp.tile([P, EC, 512], FP8, tag=f"xs8{sb}")
                            nc.sync.dma_start(out=xt8, in_=x8_r[:, :, sl])
                            xs8.append(xt8)
                            xte = xsp.tile([P, EC, 512], FP8, tag=f"xse{sb}")
                            nc.sync.dma_start(out=xte, in_=xe_r[:, :, sl])
                            xse.append(xte)

                    load_x(0)
                    emit_w(0)
                    for sb in range(1, NSB):
                        load_x(sb)
                    wo = wop.tile([P, H, E], BF16)
                    nc.sync.dma_start(out=wo, in_=wo_r[:, :, :])

                    for c in proj_chunks(0):
                        c()
                    fin = None  # previous unit's finisher
                    for g in range(H):
                        if g + 1 < H:
                            if (g + 1) % 2 == 0:
                                emit_w((g + 1) // 2)
                            nxt = proj_chunks(g + 1)
                        else:
                            nxt = []
                        for sb in range(NSB):
                            f = att_unit(g, sb)
                            if fin is not None:
                                fin()
                            fin = f
                            for c in nxt[sb * 3:(sb + 1) * 3]:
                                c()
                            if g == H - 1 and sb > 0:
                                out_chunks_sb(sb - 1)
                        for c in nxt[12:]:
                            c()
                    fin()
                    out_chunks_sb(NSB - 1)

    nc.compile()
    return nc


_NC_CACHE = []


def _get_nc():
    if not _NC_CACHE:
        _NC_CACHE.append(build_nc())
    return _NC_CACHE[0]


def make_host_inputs(hidden_state, Wq, bq, Wk, bk, Wv, bv, Wo, bo, qk8=False):
    """Common host-side preprocessing: transpose + bf16/fp8 conversion.

    bk is mathematically a no-op through softmax (per-query constant shift).
    """
    F8 = ml_dtypes.float8_e4m3fn
    WqT = np.ascontiguousarray(np.asarray(Wq, np.float32).reshape(E, E).T.astype(BF_NP))
    WkT = np.ascontiguousarray(np.asarray(Wk, np.float32).reshape(E, E).T.astype(BF_NP))
    # qk8 scaling: Wq/Wk/Wv (and bq/bv) x WSC so the fp8 weight residuals
    # escape the e4m3 subnormal floor; Wo / WSC compensates the V side, the
    # exp scale compensates Q.K.
    WvT = np.ascontiguousarray(
        (np.asarray(Wv, np.float32).reshape(E, E).T * WSC).astype(BF_NP))
    WoT = np.ascontiguousarray((np.asarray(Wo, np.float32).T / WSC).astype(BF_NP))

    def split8(wT):
        ws = (wT.astype(np.float32) * WSC).astype(BF_NP)
        w8 = ws.astype(F8)
        we = (ws.astype(np.float32) - w8.astype(np.float32)).astype(F8)
        return w8, we

    Wq8, Wqe = split8(WqT)
    Wk8, Wke = split8(WkT)
    bq2a = np.ascontiguousarray(
        np.asarray(bq, np.float32).reshape(H, D).T * (WSC if qk8 else 1.0))
    bv2a = np.ascontiguousarray(np.asarray(bv, np.float32).reshape(H, D).T * WSC)
    bob = np.ascontiguousarray(np.tile(
        np.asarray(bo, np.float32).reshape(1, E), (P, 1)).astype(BF_NP))
    hs = np.asarray(hidden_state, np.float32)
    in_maps = []
    for c in range(B):
        xTb = np.ascontiguousarray(hs[c].T.astype(BF_NP))
        xT8 = xTb.astype(F8)
        xTe = (xTb.astype(np.float32) - xT8.astype(np.float32)).astype(F8)
        in_maps.append({
            "xT": xTb, "xT8": xT8, "xTe": xTe,
            "WqT": WqT, "WkT": WkT, "WvT": WvT, "WoT": WoT,
            "Wq8T": Wq8, "WqeT": Wqe, "Wk8T": Wk8, "WkeT": Wke,
            "bq2": bq2a, "bv2": bv2a, "bob": bob,
        })
    return in_maps


def kernel(hidden_state, Wq, bq, Wk, bk, Wv, bv, Wo, bo):
    in_maps = make_host_inputs(hidden_state, Wq, bq, Wk, bk, Wv, bv, Wo, bo)
    nc = _get_nc()
    res = run_bass_kernel_spmd(nc, in_maps, core_ids=list(range(B)))
    return np.stack([res.results[c]["out"] for c in range(B)])
